# revision 1
# baseline (speedup 1.0000x reference)
"""GQA attention layer (B=2, S=2048, H=2048, 16 Q heads / 4 KV heads, RoPE,
causal softmax) on 8 Trainium2 NeuronCores.

Sharding: core = b * 4 + kv  (batch b in {0,1}, kv head in {0..3}).
Each core computes the 4 Q heads of one (batch, kv-group):
  - qT/kT projections from pre-transposed hidden states (host supplies hsT);
    v projected directly into natural [s, d] layout (lhsT/rhs swapped)
  - RoPE applied in transposed layout via a signed permutation matmul
  - attention entirely in transposed orientation:
      scoresT[k, q] = K @ Q^T  (per 128-row k-chunk, causal-skipped, and
      diagonal chunks restricted to their valid q-window, min width 256
      so f32r matmuls keep the 1 cycle/row rate)
      exp on ACT engine (no max subtraction -- scores are O(6))
      attT = V^T @ expT accumulated in PSUM, row-sums via ones-matmul,
      normalization by broadcast reciprocal (K=1 outer-product matmul)
  - o_proj partial = attT.T @ Wo[rows of this kv group]
Host sums the 4 partial outputs per batch (the "all-reduce").

I/O in bf16, device-side math in f32r/f32.  Weights stream on the ACT HWDGE
queue in parallel with hidden states on the SP queue so the first projection
matmul issues at ~7us instead of ~27us.
"""

import math

import numpy as np
import ml_dtypes

import concourse.bass as bass
import concourse.mybir as mybir
import concourse.tile as tile
from concourse import bacc
from concourse.bass_utils import run_bass_kernel_spmd

F32 = mybir.dt.float32
F32R = mybir.dt.float32r
BF16 = mybir.dt.bfloat16
NP_BF16 = ml_dtypes.bfloat16

B = 2
S = 2048
H = 2048
D = 128
N_HEADS = 16
N_KV = 4
G = 4  # q heads per kv head (= heads per core)
P = 128
SG = 512  # S processed per group
NSG = S // SG  # 4
KC = S // P  # 16 k chunks
HC = H // P  # 16 contraction chunks for projections
SCALE = 1.0 / math.sqrt(D)

# valid q-window start for diagonal chunk o (min width 256 keeps f32r fast)
_DIAG_Q0 = [0, 128, 256, 256]


def _build_module(repeat=1):
    nc = bacc.Bacc(
        "TRN2",
        target_bir_lowering=False,
        debug=False,
        enable_asserts=False,
        num_devices=8,
    )

    hsT = nc.dram_tensor("hsT", [H, S], BF16, kind="ExternalInput").ap()
    wq = nc.dram_tensor("wq", [H, G * D], BF16, kind="ExternalInput").ap()
    wk = nc.dram_tensor("wk", [H, D], BF16, kind="ExternalInput").ap()
    wv = nc.dram_tensor("wv", [H, D], BF16, kind="ExternalInput").ap()
    wo = nc.dram_tensor("wo", [G * D, H], BF16, kind="ExternalInput").ap()
    cosT = nc.dram_tensor("cosT", [D, S], BF16, kind="ExternalInput").ap()
    sinT = nc.dram_tensor("sinT", [D, S], BF16, kind="ExternalInput").ap()
    trimask = nc.dram_tensor("trimask", [P, 2 * P], BF16, kind="ExternalInput").ap()
    rotm = nc.dram_tensor("rotm", [D, D], F32, kind="ExternalInput").ap()
    ones = nc.dram_tensor("ones", [P, P], BF16, kind="ExternalInput").ap()
    ident = nc.dram_tensor("ident", [P, P], BF16, kind="ExternalInput").ap()
    y = nc.dram_tensor("y", [S, H], BF16, kind="ExternalOutput").ap()

    with tile.TileContext(nc) as tc:
        args = (tc, hsT, wq, wk, wv, wo, cosT, sinT, trimask, rotm, ones, ident, y)
        if repeat == 1:
            _kernel_body(*args)
        else:
            # timing-only variant: run the body `repeat` times inside the NEFF
            # so device time dominates the per-dispatch overhead
            with tc.For_i(0, repeat, 1):
                _kernel_body(*args)
    nc.compile()
    return nc


def _kernel_body(tc, hsT, wq, wk, wv, wo, cosT, sinT, trimask, rotm, ones, ident, y):
    nc = tc.nc
    exp_f = mybir.ActivationFunctionType.Exp

    y_t = y.rearrange("(m p) n -> m p n", p=P)

    with (
        tc.tile_pool(name="persist", bufs=1) as persist,
        tc.tile_pool(name="const", bufs=1) as constp,
    ):
        qsb = persist.tile([P, G, S], F32R, name="qsb", tag="qsb")  # q^T rope'd
        ksb = persist.tile([P, S], F32R, name="ksb", tag="ksb")  # k^T rope'd
        vsb = persist.tile([P, KC, D], BF16, name="vsb", tag="vsb")  # v natural
        wo_sb = persist.tile([P, G, H], BF16, name="wo", tag="wo")

        rot_sb = constp.tile([D, D], F32R, name="rot", tag="rot")
        ones_sb = constp.tile([P, P], BF16, name="ones", tag="ones")
        ident_sb = constp.tile([P, P], BF16, name="ident", tag="ident")
        mask_sb = constp.tile([P, 2 * P], BF16, name="mask", tag="mask")

        # weights + constants stream on the ACT HWDGE queue; hidden states on
        # the SP queue.  First projection matmul needs wq + hst2[sg=0] only.
        with (
            tc.tile_pool(name="weights1", bufs=1) as wpool,
            tc.tile_pool(name="hst", bufs=1) as hpool,
        ):
            wq_sb = wpool.tile([P, HC, G * D], BF16, name="wq", tag="wq")
            wk_sb = wpool.tile([P, HC, D], BF16, name="wk", tag="wk")
            wv_sb = wpool.tile([P, HC, D], BF16, name="wv", tag="wv")
            cos_sb = wpool.tile([D, S], BF16, name="cos", tag="cos")
            sin_sb = wpool.tile([D, S], BF16, name="sin", tag="sin")
            # wq + hst2[sg0] stream as interleaved hc-quarters on the SP
            # queue so the first q matmul can issue after ~2us; everything
            # else goes on the ACT HWDGE queue (whose head also carries the
            # exp LoadActFuncSet).
            hst2 = hpool.tile([P, HC, S], BF16, name="hst2", tag="hst2")
            hsT_r = hsT.rearrange("(hc p) s -> p hc s", p=P)
            wq_r = wq.rearrange("(hc p) c -> p hc c", p=P)
            for qq in range(4):
                hsl = slice(qq * (HC // 4), (qq + 1) * (HC // 4))
                nc.sync.dma_start(wq_sb[:, hsl, :], wq_r[:, hsl, :])
                nc.sync.dma_start(hst2[:, hsl, 0:SG], hsT_r[:, hsl, 0:SG])
            nc.scalar.dma_start(wk_sb[:], wk.rearrange("(hc p) c -> p hc c", p=P))
            nc.scalar.dma_start(wv_sb[:], wv.rearrange("(hc p) c -> p hc c", p=P))
            nc.scalar.dma_start(rot_sb[:], rotm.bitcast(F32R))
            nc.scalar.dma_start(ones_sb[:], ones)
            nc.scalar.dma_start(ident_sb[:], ident)
            nc.scalar.dma_start(mask_sb[:], trimask)
            nc.scalar.dma_start(cos_sb[:], cosT)
            nc.scalar.dma_start(sin_sb[:], sinT)
            nc.scalar.dma_start(wo_sb[:], wo.rearrange("(rc p) n -> p rc n", p=P))
            for sg in range(1, NSG):
                ssl = slice(sg * SG, (sg + 1) * SG)
                nc.sync.dma_start(hst2[:, :, ssl], hsT_r[:, :, ssl])

            # ---------------- Phase 1: projections + RoPE -----------------
            # Per sg: q-block, k-block, v-block matmuls.  PSUM->SBUF raw
            # copies are emitted right after each block so the ACT engine
            # drains them under the next block's matmuls, leaving ACT free
            # for attention exps at the phase boundary.  RoPE elementwise
            # work is split across engines: t1=raw*cos on Pool, t2=rot*sin
            # on DVE, final add on Pool.
            with (
                tc.tile_pool(name="p1psum", bufs=1, space="PSUM") as ppool,
                tc.tile_pool(name="p1raw", bufs=5) as rawpool,
                tc.tile_pool(name="p1tmp", bufs=2) as tpool,
                tc.tile_pool(name="p1rot", bufs=1, space="PSUM") as rpool,
            ):
                def rope_finish(dst, raw, ssl):
                    rps = rpool.tile([P, SG], F32, name="rotp", tag="rotp")
                    nc.tensor.matmul(rps[:], rot_sb[:], raw[:], start=True, stop=True)
                    t1 = tpool.tile([P, SG], F32, name="t1", tag="t1")
                    nc.vector.tensor_mul(t1[:], raw[:], cos_sb[:, ssl])
                    t2 = tpool.tile([P, SG], F32, name="t2", tag="t2")
                    nc.vector.tensor_mul(t2[:], rps[:], sin_sb[:, ssl])
                    nc.vector.tensor_add(dst, t1[:], t2[:])

                for sg in range(NSG):
                    ssl = slice(sg * SG, (sg + 1) * SG)
                    q_ps = [
                        ppool.tile([P, SG], F32, name=f"qps{qt}", tag=f"qps{qt}")
                        for qt in range(G)
                    ]
                    k_ps = ppool.tile([P, SG], F32, name="kps", tag="kps")
                    v_ps = ppool.tile([P, SG], F32, name="vps", tag="vps")
                    for hc in range(HC):
                        for qt in range(G):
                            nc.tensor.matmul(
                                q_ps[qt][:],
                                wq_sb[:, hc, qt * D : (qt + 1) * D],
                                hst2[:, hc, ssl],
                                start=hc == 0,
                                stop=hc == HC - 1,
                            )
                    raws = []
                    for qt in range(G):  # ACT drains these under the k-block
                        raw = rawpool.tile([P, SG], F32R, name="rawq", tag="rawq")
                        nc.scalar.copy(raw[:], q_ps[qt][:])
                        raws.append(raw)
                    for hc in range(HC):
                        nc.tensor.matmul(
                            k_ps[:], wk_sb[:, hc, :], hst2[:, hc, ssl],
                            start=hc == 0, stop=hc == HC - 1,
                        )
                    rawk = rawpool.tile([P, SG], F32R, name="rawk", tag="rawq")
                    nc.scalar.copy(rawk[:], k_ps[:])
                    # v projected transposed like k, then PE-transposed to
                    # natural layout in bf16 (fewer, bigger matmuls than a
                    # direct natural-layout projection -- per-instruction
                    # overhead dominates 128-row matmuls on hardware)
                    for hc in range(HC):
                        nc.tensor.matmul(
                            v_ps[:], wv_sb[:, hc, :], hst2[:, hc, ssl],
                            start=hc == 0, stop=hc == HC - 1,
                        )
                    rawv = tpool.tile([P, SG], BF16, name="rawv", tag="rawv")
                    nc.scalar.copy(rawv[:], v_ps[:])
                    for c4 in range(SG // P):
                        tp = rpool.tile([P, P], BF16, name="vtp", tag="vtp")
                        nc.tensor.transpose(
                            tp[:], rawv[:, c4 * P : (c4 + 1) * P], ident_sb[:]
                        )
                        nc.scalar.copy(vsb[:, sg * (SG // P) + c4, :], tp[:])
                    for qt in range(G):
                        rope_finish(qsb[:, qt, ssl], raws[qt], ssl)
                    rope_finish(ksb[:, ssl], rawk, ssl)

        # -------- Phase 2: attention (g-outer) with interleaved o_proj -----
        # The o_proj ng-groups of q-block g-1 are emitted as PE "fillers"
        # between the attention steps of q-block g, so exp latency and the
        # softmax-tail reciprocal never leave the PE idle.
        with (
            tc.tile_pool(name="mixps", bufs=3, space="PSUM") as mixpool,
            tc.tile_pool(name="avps", bufs=2, space="PSUM") as avpool,
            tc.tile_pool(name="rsps", bufs=2, space="PSUM") as rspool,
            tc.tile_pool(name="expt", bufs=5) as expool,
            tc.tile_pool(name="small", bufs=4) as small,
            tc.tile_pool(name="osb", bufs=2) as osb,
        ):
            attT = persist.tile([P, G, S], BF16, name="attT", tag="attT")

            def make_fillers(g):
                """One closure per o_proj ng-group for the m-blocks of g."""
                fillers = []
                for m in range(4 * g, 4 * g + 4):
                    for ng in range(H // SG):
                        def emit(m=m, ng=ng):
                            o_ps = mixpool.tile([P, SG], F32, name="o", tag="mix")
                            for rc in range(G):
                                nc.tensor.matmul(
                                    o_ps[:],
                                    attT[:, rc, m * P : (m + 1) * P],
                                    wo_sb[:, rc, ng * SG : (ng + 1) * SG],
                                    start=rc == 0,
                                    stop=rc == G - 1,
                                )
                            nsl = slice(ng * SG, (ng + 1) * SG)
                            ot = osb.tile([P, SG], BF16, name="ot", tag="ot")
                            nc.vector.tensor_copy(ot[:], o_ps[:])
                            nc.sync.dma_start(y_t[m][:, nsl], ot[:])
                        fillers.append(emit)
                return fillers

            def attn_head(g, h, av_ps, rs_ps, fillers):
                """scores+exp+av+rowsum for head h of q-block g.

                Software-pipelined: scores for later steps are emitted ahead
                of the av/rs of earlier steps, and one o_proj filler group
                slots in per step, so the PE never waits on an exp."""
                qsl = slice(g * SG, (g + 1) * SG)
                nk = 4 * g + 4

                def emit_scores(c):
                    if c < 4 * g:  # full chunk
                        sc = mixpool.tile([P, SG], F32, name="sc", tag="mix")
                        nc.tensor.matmul(
                            sc[:],
                            ksb[:, c * P : (c + 1) * P],
                            qsb[:, h, qsl],
                            start=True,
                            stop=True,
                        )
                        ex = expool.tile([P, SG], BF16, name="ex", tag="ex")
                        nc.scalar.activation(ex[:], sc[:], exp_f, scale=SCALE)
                        return ex
                    # diagonal chunk, restricted to its valid q-window
                    o = c - 4 * g
                    q0 = _DIAG_Q0[o]
                    wsl = slice(q0, SG)
                    sc = mixpool.tile([P, SG], F32, name="sc", tag="mix")
                    nc.tensor.matmul(
                        sc[:, wsl],
                        ksb[:, c * P : (c + 1) * P],
                        qsb[:, h, g * SG + q0 : (g + 1) * SG],
                        start=True,
                        stop=True,
                    )
                    ex = expool.tile([P, SG], BF16, name="ex", tag="ex")
                    nc.scalar.activation(ex[:, wsl], sc[:, wsl], exp_f, scale=SCALE)
                    # triangle mask on the leading window (q0 .. o*128+128),
                    # on the otherwise-idle Pool engine
                    mw = o * P + P - q0
                    nc.vector.tensor_mul(
                        ex[:, q0 : o * P + P],
                        ex[:, q0 : o * P + P],
                        mask_sb[:, 2 * P - mw :],
                    )
                    return ex

                def win(c):
                    return slice(0 if c < 4 * g else _DIAG_Q0[c - 4 * g], SG)

                def emit_av(c, ex):
                    w = win(c)
                    nc.tensor.matmul(
                        av_ps[:, w], vsb[:, c, :], ex[:, w],
                        start=c == 0, stop=c == nk - 1,
                    )
                    nc.tensor.matmul(
                        rs_ps[:, w], ones_sb[:, 0:1], ex[:, w],
                        start=c == 0, stop=c == nk - 1,
                    )

                depth = 1 if fillers else 2
                inflight = []
                for c in range(nk):
                    inflight.append((c, emit_scores(c)))
                    if c % 2 == 1 and len(fillers) > 1:
                        # pace the o_proj fillers evenly over this g's pairs
                        pace[0] += 1
                        while (
                            len(fillers) > 1
                            and (pace[1] - len(fillers) + 1) * pace[2]
                            < pace[0] * pace[1]
                        ):
                            fillers.pop(0)()
                    if len(inflight) > depth:
                        emit_av(*inflight.pop(0))
                for item in inflight:
                    emit_av(*item)

            def head_tail(g, h, av_ps, rs_ps, filler=None):
                """reciprocal -> broadcast -> normalize into attT."""
                qsl = slice(g * SG, (g + 1) * SG)
                rec = small.tile([1, SG], F32R, name="rec", tag="rec")
                with nc.allow_low_precision(reason="softmax denom recip to f32r"):
                    nc.vector.reciprocal(rec[:], rs_ps[:])
                if filler is not None:  # PE works while the reciprocal runs
                    filler()
                bc_sb = small.tile([P, SG], F32R, name="bcs", tag="bcs")
                nc.gpsimd.partition_broadcast(bc_sb[:], rec[:])
                nc.vector.tensor_mul(attT[:, h, qsl], av_ps[:], bc_sb[:])

            for g in range(NSG):
                fillers = make_fillers(g - 1) if g > 0 else []
                pace = [0, len(fillers), G * (2 * g + 2)]
                pend = []  # (h, av_ps, rs_ps) with tail not yet emitted
                for h in range(G):
                    av_ps = avpool.tile([P, SG], F32, name="av", tag="av")
                    rs_ps = rspool.tile([1, SG], F32, name="rs", tag="rs")
                    attn_head(g, h, av_ps, rs_ps, fillers)
                    pend.append((h, av_ps, rs_ps))
                    if len(pend) == 2:
                        # emit tail for the OLDER pending head so its
                        # reciprocal latency hides under this head's matmuls
                        head_tail(g, *pend.pop(0))
                head_tail(
                    g,
                    *pend.pop(0),
                    filler=fillers.pop(0) if fillers else None,
                )
                for f in fillers:
                    f()

            # last q-block's o_proj has no following attention to hide in
            for f in make_fillers(NSG - 1):
                f()


def _host_constants():
    inv_freq = 1.0 / (10000.0 ** (np.arange(0, D, 2, dtype=np.float32) / D))
    t = np.arange(S, dtype=np.float32)
    freqs = np.outer(t, inv_freq)
    emb = np.concatenate([freqs, freqs], -1)  # [S, D]
    cosT = np.ascontiguousarray(np.cos(emb).T).astype(NP_BF16)
    sinT = np.ascontiguousarray(np.sin(emb).T).astype(NP_BF16)
    rot = np.zeros((D, D), np.float32)  # rot(q)^T = M @ q^T ; lhsT = M.T
    for i in range(D // 2):
        rot[i, i + D // 2] = -1.0
        rot[i + D // 2, i] = 1.0
    rot_lhsT = np.ascontiguousarray(rot.T)
    # trimask[:, 128:256] = lower-triangle (dk <= dq); [:, 0:128] = 0
    tri = np.zeros((P, 2 * P), np.float32)
    dk = np.arange(P)[:, None]
    dq = np.arange(P)[None, :]
    tri[:, P:] = (dk <= dq).astype(np.float32)
    tri = tri.astype(NP_BF16)
    ones = np.ones((P, P), np.float32).astype(NP_BF16)
    ident = np.eye(P, dtype=np.float32).astype(NP_BF16)
    return cosT, sinT, rot_lhsT, tri, ones, ident


_NC_CACHE = None


def _get_module():
    global _NC_CACHE
    if _NC_CACHE is None:
        _NC_CACHE = _build_module()
    return _NC_CACHE


def _make_in_maps(hidden_states, Wq, Wk, Wv, Wo):
    cosT, sinT, rot_lhsT, tri, ones, ident = _host_constants()
    in_maps = []
    for core in range(8):
        b, kv = core // 4, core % 4
        in_maps.append(
            {
                "hsT": np.ascontiguousarray(hidden_states[b].T).astype(NP_BF16),
                "wq": np.ascontiguousarray(
                    Wq[:, kv * G * D : (kv + 1) * G * D]
                ).astype(NP_BF16),
                "wk": np.ascontiguousarray(Wk[:, kv * D : (kv + 1) * D]).astype(
                    NP_BF16
                ),
                "wv": np.ascontiguousarray(Wv[:, kv * D : (kv + 1) * D]).astype(
                    NP_BF16
                ),
                "wo": np.ascontiguousarray(
                    Wo[kv * G * D : (kv + 1) * G * D, :]
                ).astype(NP_BF16),
                "cosT": cosT,
                "sinT": sinT,
                "trimask": tri,
                "rotm": rot_lhsT,
                "ones": ones,
                "ident": ident,
            }
        )
    return in_maps


def kernel(hidden_states, Wq, Wk, Wv, Wo, _trace=False, _tmpdir=None):
    hidden_states = np.asarray(hidden_states, dtype=np.float32)
    Wq = np.asarray(Wq, dtype=np.float32)
    Wk = np.asarray(Wk, dtype=np.float32)
    Wv = np.asarray(Wv, dtype=np.float32)
    Wo = np.asarray(Wo, dtype=np.float32)

    nc = _get_module()
    in_maps = _make_in_maps(hidden_states, Wq, Wk, Wv, Wo)

    res = run_bass_kernel_spmd(
        nc,
        in_maps,
        core_ids=list(range(8)),
        trace=_trace,
        tmpdir=_tmpdir,
        stitch_traces=False,
    )

    out = np.zeros((B, S, H), np.float32)
    for core in range(8):
        out[core // 4] += res.results[core]["y"].astype(np.float32)
    kernel._last_result = res
    return out


_BENCH_CACHE = None


def _get_bench_fn():
    """Jitted 8-core executor (no donation) reusable across calls, for
    correctness + repeated-execute timing. Mirrors bass2jax.run_bass_via_pjrt."""
    global _BENCH_CACHE
    if _BENCH_CACHE is not None:
        return _BENCH_CACHE
    import jax
    from jax.sharding import Mesh, PartitionSpec
    from jax.experimental.shard_map import shard_map
    import concourse.mybir as _mybir
    from concourse import bass2jax

    nc = _get_module()
    bass2jax.install_neuronx_cc_hook()
    partition_name = (
        nc.partition_id_tensor.name if nc.partition_id_tensor else None
    )
    in_names, out_names, out_avals = [], [], []
    for alloc in nc.m.functions[0].allocations:
        if not isinstance(alloc, _mybir.MemoryLocationSet):
            continue
        name = alloc.memorylocations[0].name
        if alloc.kind == "ExternalInput":
            if name != partition_name:
                in_names.append(name)
        elif alloc.kind == "ExternalOutput":
            out_names.append(name)
            out_avals.append(
                jax.core.ShapedArray(
                    tuple(alloc.tensor_shape), _mybir.dt.np(alloc.dtype)
                )
            )
    all_names = list(in_names) + list(out_names)
    if partition_name is not None:
        all_names.append(partition_name)

    def _body(*args):
        operands = list(args)
        if partition_name is not None:
            operands.append(bass2jax.partition_id_tensor())
        outs = bass2jax._bass_exec_p.bind(
            *operands,
            out_avals=tuple(out_avals),
            in_names=tuple(all_names),
            out_names=tuple(out_names),
            lowering_input_output_aliases=(),
            sim_require_finite=True,
            sim_require_nnan=True,
            nc=nc,
        )
        return tuple(outs)

    devices = jax.devices()[:8]
    mesh = Mesh(np.asarray(devices), ("core",))
    n_in = len(in_names)
    n_out = len(out_names)
    sharded = jax.jit(
        shard_map(
            _body,
            mesh=mesh,
            in_specs=(PartitionSpec("core"),) * (n_in + n_out),
            out_specs=(PartitionSpec("core"),) * n_out,
            check_rep=False,
        ),
        keep_unused=True,
    )
    _BENCH_CACHE = (sharded, in_names, out_names, out_avals)
    return _BENCH_CACHE


def benchmark(hidden_states, Wq, Wk, Wv, Wo, iters=30):
    """Returns (full_output, per_iter_ns)."""
    import time as _time

    import jax
    from jax.sharding import Mesh, NamedSharding, PartitionSpec

    sharded, in_names, out_names, out_avals = _get_bench_fn()
    in_maps = _make_in_maps(
        np.asarray(hidden_states, np.float32),
        np.asarray(Wq, np.float32),
        np.asarray(Wk, np.float32),
        np.asarray(Wv, np.float32),
        np.asarray(Wo, np.float32),
    )
    concat_in = [
        np.concatenate([in_maps[c][n] for c in range(8)], axis=0) for n in in_names
    ]
    concat_zero = [
        np.zeros((8 * a.shape[0], *a.shape[1:]), a.dtype) for a in out_avals
    ]
    mesh = Mesh(np.asarray(jax.devices()[:8]), ("core",))
    sharding = NamedSharding(mesh, PartitionSpec("core"))
    args = [jax.device_put(a, sharding) for a in concat_in + concat_zero]
    out = sharded(*args)  # compile + first exec
    jax.block_until_ready(out)
    for _ in range(3):
        jax.block_until_ready(sharded(*args))
    t0 = _time.perf_counter()
    last = None
    for _ in range(iters):
        last = sharded(*args)
    jax.block_until_ready(last)
    per_iter_ns = (_time.perf_counter() - t0) / iters * 1e9

    full = np.zeros((B, S, H), np.float32)
    yi = out_names.index("y")
    yall = np.asarray(out[yi]).reshape(8, S, H).astype(np.float32)
    for core in range(8):
        full[core // 4] += yall[core]
    return full, per_iter_ns


if __name__ == "__main__":
    x = {
        "hidden_states": np.random.randn(B, S, H).astype(np.float32),
        "Wq": np.random.randn(H, H).astype(np.float32) * 0.02,
        "Wk": np.random.randn(H, N_KV * D).astype(np.float32) * 0.02,
        "Wv": np.random.randn(H, N_KV * D).astype(np.float32) * 0.02,
        "Wo": np.random.randn(H, H).astype(np.float32) * 0.02,
    }
    y = kernel(**x)
    print("ran, out shape", y.shape)

